# revision 11
# baseline (speedup 1.0000x reference)
"""VQ-codebook kernel for Trainium2 (Bass/Tile), 8 NeuronCores data-parallel.

Computes, for x:[512,365,10], mask:[512,365], prototypes:[64,365,10]:
    d[b,p]  = sum_{t,c} mask[b,t] * (x[b,t,c] - proto[p,t,c])^2
    idx[b]  = argmin_p d[b,p]
    out_seq = prototypes[idx]
and returns the reference's 6-tuple (out_seq, x, d, idx, label, mask).

Strategy (per core, B sharded 512 -> 8x64):
  d = x2[b] - 2*xp[b,p] + p2[b,p], all via fp32 PE matmuls with the
  contraction dim (t*c = 3650, zero-padded to 3712 = 29*128) on partitions:
    - chain1: lhsT = (mask*x)^T chunks, rhs = [-2*proto^T | x^T] -> S[b, 0:64]
      accumulates -2*xp, S[b, 64:128] accumulates the Gram matrix G whose
      diagonal is x2.
    - chain2: lhsT = mask10^T chunks, rhs = (proto^T)^2 -> accumulates p2
      into S[b, 0:64].
  x2 = diag(G) via tensor_tensor_reduce with the identity; d = S[:,0:64]+x2.
  argmin via vector.max/max_index on -d; gather via exact one-hot fp32
  matmul: onehot^T[p,b] = (argmin[b] == p), out_seq = onehot^T.T @ proto.
  (indirect-DMA gather crashes the device on this runtime path: probe.py
  reproduced NRT_EXEC_UNIT_UNRECOVERABLE with indirect_dma_start alone.)

Transposed operands are packed on the host (pure layout, no arithmetic):
DRAM [128, 29, n] so each SBUF partition loads linearly.
"""

import os
import sys

import numpy as np

for _p in ("/opt/trn_rl_repo",):
    if os.path.isdir(_p) and _p not in sys.path:
        sys.path.insert(0, _p)

B, T, C, P = 512, 365, 10, 64
NCORES = 8
BL = B // NCORES  # 64 samples per core
TC = T * C  # 3650
KCH = 29  # contraction chunks of 128
KPAD = KCH * 128  # 3712

_PROG = None


def _build_program():
    import concourse.bacc as bacc
    import concourse.mybir as mybir
    import concourse.tile as tile
    from concourse import bass

    f32 = mybir.dt.float32
    nc = bacc.Bacc("TRN2", target_bir_lowering=False, debug=False,
                   num_devices=NCORES)

    xt_d = nc.dram_tensor("xt", [128, KCH, BL], f32, kind="ExternalInput")
    mt_d = nc.dram_tensor("mt10", [128, KCH, BL], f32, kind="ExternalInput")
    pt_d = nc.dram_tensor("protot", [128, KCH, P], f32, kind="ExternalInput")
    pr_d = nc.dram_tensor("proto", [P, TC], f32, kind="ExternalInput")
    id_d = nc.dram_tensor("ident", [BL, BL], f32, kind="ExternalInput")
    io_d = nc.dram_tensor("iotac", [BL, 1], f32, kind="ExternalInput")
    oseq_d = nc.dram_tensor("outseq", [BL, TC], f32, kind="ExternalOutput")
    dist_d = nc.dram_tensor("dist", [BL, P], f32, kind="ExternalOutput")
    idx_d = nc.dram_tensor("idx", [BL], mybir.dt.int32, kind="ExternalOutput")

    NW = P + BL  # 128 columns per chain1 rhs chunk

    with tile.TileContext(nc) as tc:
        with tc.tile_pool(name="sb", bufs=1) as sb, \
             tc.tile_pool(name="ps", bufs=1, space="PSUM") as ps, \
             tc.tile_pool(name="ps2", bufs=4, space="PSUM") as ps2:
            rhs_all = sb.tile([128, KCH * NW], f32)
            maskt = sb.tile([128, KCH * BL], f32)
            protot = sb.tile([128, KCH * P], f32)
            psq = sb.tile([128, KCH * P], f32)
            mxt = sb.tile([128, KCH * BL], f32)
            ident = sb.tile([BL, BL], f32)
            iotac = sb.tile([BL, 1], f32)
            pr_sb = sb.tile([P, TC], f32)

            rhs3 = rhs_all[:].rearrange("r (k n) -> r k n", n=NW)

            nc.sync.dma_start(out=rhs3[:, :, P:NW], in_=xt_d[:, :, :])
            nc.sync.dma_start(out=iotac[:], in_=io_d[:, :])
            nc.sync.dma_start(out=pr_sb[:], in_=pr_d[:, :])
            nc.sync.dma_start(
                out=maskt[:].rearrange("r (k n) -> r k n", n=BL),
                in_=mt_d[:, :, :])
            nc.sync.dma_start(
                out=protot[:].rearrange("r (k n) -> r k n", n=P),
                in_=pt_d[:, :, :])
            nc.sync.dma_start(out=ident[:], in_=id_d[:, :])

            # rhs left half = -2 * proto^T ; psq = (proto^T)^2
            nc.scalar.mul(
                out=rhs3[:, :, 0:P],
                in_=protot[:].rearrange("r (k n) -> r k n", n=P),
                mul=-2.0)
            nc.scalar.square(out=psq[:], in_=protot[:])

            # mx^T = x^T * mask10^T
            nc.vector.tensor_tensor(
                out=mxt[:].rearrange("r (k n) -> r k n", n=BL),
                in0=rhs3[:, :, P:NW],
                in1=maskt[:].rearrange("r (k n) -> r k n", n=BL),
                op=mybir.AluOpType.mult)

            # chain1: S = [-2*xp | G], clean accumulation group
            S = ps.tile([BL, NW], f32)
            for k in range(KCH):
                nc.tensor.matmul(
                    out=S[:, :],
                    lhsT=mxt[:, k * BL:(k + 1) * BL],
                    rhs=rhs_all[:, k * NW:(k + 1) * NW],
                    start=(k == 0), stop=(k == KCH - 1))
            # chain2: S2 = p2 = mask10 @ protosq^T, its own group
            S2 = ps.tile([BL, P], f32)
            for k in range(KCH):
                nc.tensor.matmul(
                    out=S2[:, :],
                    lhsT=maskt[:, k * BL:(k + 1) * BL],
                    rhs=psq[:, k * P:(k + 1) * P],
                    start=(k == 0), stop=(k == KCH - 1))

            # evict PSUM via plain copies only (ttr/tensor_scalar reading
            # PSUM crash the device on this runtime path — see bisect2.py)
            s_sb = sb.tile([BL, NW], f32)
            s2_sb = sb.tile([BL, P], f32)
            nc.vector.tensor_copy(out=s_sb[:], in_=S[:, :])
            nc.vector.tensor_copy(out=s2_sb[:], in_=S2[:, :])

            # x2 = diag(G) = rowsum(G * I)
            gscratch = sb.tile([BL, BL], f32)
            x2col = sb.tile([BL, 1], f32)
            nc.vector.tensor_tensor(
                out=gscratch[:], in0=s_sb[:, P:NW], in1=ident[:],
                op=mybir.AluOpType.mult)
            nc.vector.tensor_reduce(
                out=x2col[:], in_=gscratch[:],
                axis=mybir.AxisListType.X, op=mybir.AluOpType.add)

            # d = (-2xp + x2) + p2 ; negd = -d
            t1 = sb.tile([BL, P], f32)
            d_sb = sb.tile([BL, P], f32)
            negd = sb.tile([BL, P], f32)
            nc.vector.tensor_scalar(
                out=t1[:], in0=s_sb[:, 0:P],
                scalar1=x2col[:, 0:1], scalar2=None,
                op0=mybir.AluOpType.add)
            nc.vector.tensor_tensor(
                out=d_sb[:], in0=t1[:], in1=s2_sb[:],
                op=mybir.AluOpType.add)
            nc.vector.tensor_scalar(
                out=negd[:], in0=d_sb[:],
                scalar1=-1.0, scalar2=None,
                op0=mybir.AluOpType.mult)

            # argmin d = first index of max(-d)
            vmax = sb.tile([BL, 8], f32)
            idx8 = sb.tile([BL, 8], mybir.dt.uint32)
            idxi = sb.tile([BL, 1], mybir.dt.int32)
            idxf = sb.tile([BL, 1], f32)
            nc.vector.max(vmax[:], negd[:])
            nc.vector.max_index(idx8[:], vmax[:], negd[:])
            nc.vector.tensor_copy(out=idxi[:], in_=idx8[:, 0:1])
            nc.vector.tensor_copy(out=idxf[:], in_=idx8[:, 0:1])

            # one-hot^T[p, b] = (argmin[b] == p), built via PE transpose
            # of the argmin column + K=1 broadcast matmul across partitions
            ps_row = ps.tile([1, BL], f32)
            nc.tensor.matmul(out=ps_row[:], lhsT=idxf[:], rhs=ident[:],
                             start=True, stop=True)
            row_sb = sb.tile([1, BL], f32)
            nc.vector.tensor_copy(out=row_sb[:], in_=ps_row[:])
            ones_row = sb.tile([1, BL], f32)
            nc.vector.memset(ones_row[:], 1.0)
            ps_bc = ps.tile([P, BL], f32)
            nc.tensor.matmul(out=ps_bc[:], lhsT=ones_row[:], rhs=row_sb[:],
                             start=True, stop=True)
            bc_sb = sb.tile([P, BL], f32)
            nc.vector.tensor_copy(out=bc_sb[:], in_=ps_bc[:])
            onehot_t = sb.tile([P, BL], f32)
            nc.vector.tensor_scalar(
                out=onehot_t[:], in0=bc_sb[:],
                scalar1=iotac[:, 0:1], scalar2=None,
                op0=mybir.AluOpType.is_equal)

            # out_seq = onehot^T.T @ proto  (exact: one 1.0 per column)
            gath = sb.tile([BL, TC], f32)
            NGC = 512
            for j in range((TC + NGC - 1) // NGC):
                n0 = j * NGC
                n1 = min(n0 + NGC, TC)
                ps_g = ps2.tile([BL, NGC], f32, tag="ps_g")
                nc.tensor.matmul(out=ps_g[:, 0:n1 - n0], lhsT=onehot_t[:],
                                 rhs=pr_sb[:, n0:n1], start=True, stop=True)
                nc.vector.tensor_copy(out=gath[:, n0:n1],
                                      in_=ps_g[:, 0:n1 - n0])

            nc.sync.dma_start(out=oseq_d[:, :], in_=gath[:])
            nc.sync.dma_start(out=dist_d[:, :], in_=d_sb[:])
            nc.sync.dma_start(out=idx_d[:, None], in_=idxi[:])

    nc.compile()
    return nc


def _get_program():
    global _PROG
    if _PROG is None:
        _PROG = _build_program()
    return _PROG


def _pack_T(a):
    """[n, TC] row-major -> transposed, zero-padded, chunk-packed [128, KCH, n]."""
    at = np.zeros((KPAD, a.shape[0]), np.float32)
    at[:TC] = a.T
    return np.ascontiguousarray(at.reshape(KCH, 128, -1).transpose(1, 0, 2))


# set by test harness: trace the next run and stash BassKernelResults here
TRACE = False
LAST_RESULTS = None


def kernel(**inputs):
    global LAST_RESULTS
    from concourse.bass_utils import run_bass_kernel_spmd

    x = np.ascontiguousarray(np.asarray(inputs["input_seq"], dtype=np.float32))
    mask = np.ascontiguousarray(np.asarray(inputs["mask"], dtype=np.float32))
    label = np.asarray(inputs["label"])
    proto = np.ascontiguousarray(
        np.asarray(inputs["prototypes"], dtype=np.float32))

    nc = _get_program()
    pr2 = np.ascontiguousarray(proto.reshape(P, TC))
    protot_packed = _pack_T(pr2)
    ident = np.eye(BL, dtype=np.float32)

    in_maps = []
    for c in range(NCORES):
        xc = x[c * BL:(c + 1) * BL].reshape(BL, TC)
        mc = np.repeat(mask[c * BL:(c + 1) * BL], C, axis=1)
        in_maps.append({
            "xt": _pack_T(xc),
            "mt10": _pack_T(mc),
            "protot": protot_packed,
            "proto": pr2,
            "ident": ident,
            "iotac": np.arange(BL, dtype=np.float32).reshape(BL, 1),
        })

    res = run_bass_kernel_spmd(nc, in_maps, list(range(NCORES)), trace=TRACE)
    LAST_RESULTS = res

    outseq = np.concatenate(
        [res.results[c]["outseq"] for c in range(NCORES)], axis=0
    ).reshape(B, T, C)
    dist = np.concatenate(
        [res.results[c]["dist"] for c in range(NCORES)], axis=0)
    idx = np.concatenate(
        [res.results[c]["idx"] for c in range(NCORES)], axis=0).astype(np.int32)

    return (outseq, x, dist, idx, label, mask)
